# revision 3
# baseline (speedup 1.0000x reference)
"""Causal self-attention (B=4, T=2048, C=1024, H=16, D=64) on 8 trn2 NeuronCores.

Sharding: core c = (batch b=c//2, head-group hg=c%2 of 8 heads / 512 channels).
Each core computes attention for its 8 heads on its batch plus the partial
output projection over its 512 channels of Wp; the host sums the two partial
projections per batch and adds bp.

Per-core layout is feature-major ("transposed"): x is sent as xT (C, T) so
q/k project directly as qT = Wq.T @ x.T with both operands k(partition)-major.
v is computed in natural (T, D) orientation with a ones-column appended per
head so that the yT = [v|1].T @ P^T matmul also yields softmax row sums.
All matmuls run as float32r (1 cyc/row on trn2, fp32 data in memory).
"""

import math

import numpy as np

B, T, C = 4, 2048, 1024
H, D = 16, 64
NCORES = 8
PAIRS = 4          # head pairs per core (2 heads = 128 channels each)
KT = C // 128      # 8 k-tiles over input channels
MT = T // 128      # 16 tiles over sequence
SC = 1.0 / math.sqrt(D)

_CACHE = {}


def _build_nc():
    from contextlib import ExitStack

    import concourse.bacc as bacc
    import concourse.mybir as mybir
    import concourse.tile as tile

    f32 = mybir.dt.float32
    f32r = mybir.dt.float32r
    AF = mybir.ActivationFunctionType

    nc = bacc.Bacc("TRN2", target_bir_lowering=False, debug=False)

    xT = nc.dram_tensor("xT", (C, T), f32r, kind="ExternalInput").ap()
    wqD = nc.dram_tensor("wq", (C, 512), f32r, kind="ExternalInput").ap()
    wkD = nc.dram_tensor("wk", (C, 512), f32r, kind="ExternalInput").ap()
    wvD = nc.dram_tensor("wv", (C, 512), f32r, kind="ExternalInput").ap()
    wpD = nc.dram_tensor("wp", (512, C), f32r, kind="ExternalInput").ap()
    bqD = nc.dram_tensor("bq", (512,), f32, kind="ExternalInput").ap()
    bkD = nc.dram_tensor("bk", (512,), f32, kind="ExternalInput").ap()
    bvD = nc.dram_tensor("bv", (512,), f32, kind="ExternalInput").ap()
    outD = nc.dram_tensor("out", (T, C), f32, kind="ExternalOutput").ap()

    with tile.TileContext(nc) as tc, ExitStack() as ctx:
        const = ctx.enter_context(tc.tile_pool(name="const", bufs=1))

        # 128x128 lower-block mask: keep (1.0) where i >= j, else 0.
        mask_tri = const.tile([128, 128], f32)
        nc.gpsimd.memset(mask_tri[:], 1.0)
        nc.gpsimd.affine_select(
            out=mask_tri[:],
            in_=mask_tri[:],
            compare_op=mybir.AluOpType.is_ge,
            fill=0.0,
            base=0,
            pattern=[[1, 128]],
            channel_multiplier=-1,
        )

        bq_sb = const.tile([128, PAIRS], f32)
        nc.sync.dma_start(bq_sb[:], bqD.rearrange("(a p) -> p a", p=128))
        bk_sb = const.tile([128, PAIRS], f32)
        nc.sync.dma_start(bk_sb[:], bkD.rearrange("(a p) -> p a", p=128))
        bv_row = const.tile([1, 512], f32)
        nc.sync.dma_start(bv_row[:], bvD.rearrange("(a n) -> a n", a=1))
        bv_bc = const.tile([128, 512], f32)
        nc.gpsimd.partition_broadcast(bv_bc[:], bv_row[:])

        # v for all heads, natural (t, d) layout, 65-wide blocks per head with
        # a ones column at index 64 (written once below; evictions fill 0:64).
        v_all = const.tile([128, MT * 8 * 65], f32r)
        v4 = v_all.rearrange("p (t h e) -> p t h e", t=MT, h=8)
        ones_sb = const.tile([128, MT * 8], f32)
        nc.gpsimd.memset(ones_sb[:], 1.0)
        nc.vector.tensor_copy(
            v4[:, :, :, 64], ones_sb.rearrange("p (t h) -> p t h", t=MT)
        )

        yT_tiles = [const.tile([128, T], f32r, name=f"yT{i}") for i in range(PAIRS)]

        # ---------------- Phase V: v = x @ Wv + bv for all 8 heads ----------
        with tc.tile_pool(name="wvp", bufs=1) as wvp, \
             tc.tile_pool(name="xv", bufs=3) as xv, \
             tc.tile_pool(name="vps", bufs=8, space="PSUM") as vps:
            wv_sb = wvp.tile([128, KT, 512], f32r)
            nc.sync.dma_start(wv_sb[:], wvD.rearrange("(k p) n -> p k n", p=128))
            for tg in range(2):
                ps = [
                    vps.tile([128, 512], f32, tag="vps", name=f"vps{tg}_{t}")
                    for t in range(8)
                ]
                for k in range(KT):
                    xk = xv.tile([128, 1024], f32r, tag="xk")
                    nc.sync.dma_start(
                        xk[:], xT[k * 128:(k + 1) * 128, tg * 1024:(tg + 1) * 1024]
                    )
                    for t8 in range(8):
                        nc.tensor.matmul(
                            ps[t8][:],
                            lhsT=(xk[:, t8 * 128:(t8 + 1) * 128]),
                            rhs=(wv_sb[:, k, :]),
                            start=(k == 0),
                            stop=(k == KT - 1),
                        )
                for t8 in range(8):
                    tt = tg * 8 + t8
                    nc.vector.tensor_add(
                        v4[:, tt, :, 0:64],
                        ps[t8].rearrange("p (h e) -> p h e", h=8),
                        bv_bc.rearrange("p (h e) -> p h e", h=8),
                    )

        qkbuf = ctx.enter_context(tc.tile_pool(name="qkbuf", bufs=2))
        wqk = ctx.enter_context(tc.tile_pool(name="wqk", bufs=2))
        xq = ctx.enter_context(tc.tile_pool(name="xq", bufs=3))
        ptp = ctx.enter_context(tc.tile_pool(name="ptp", bufs=2))
        nrm = ctx.enter_context(tc.tile_pool(name="nrm", bufs=2))

        for p in range(PAIRS):
            # ---------------- QK projections for this head pair -------------
            with tc.tile_pool(name=f"qkps{p}", bufs=8, space="PSUM") as qkps:
                wq_sb = wqk.tile([128, KT, 128], f32r, tag="wq", name=f"wq{p}")
                nc.sync.dma_start(
                    wq_sb[:],
                    wqD[:, p * 128:(p + 1) * 128].rearrange("(k q) n -> q k n", q=128),
                )
                wk_sb = wqk.tile([128, KT, 128], f32r, tag="wk", name=f"wk{p}")
                nc.sync.dma_start(
                    wk_sb[:],
                    wkD[:, p * 128:(p + 1) * 128].rearrange("(k q) n -> q k n", q=128),
                )
                qT = qkbuf.tile([128, T], f32r, tag="qT", name=f"qT{p}")
                kTt = qkbuf.tile([128, T], f32r, tag="kT", name=f"kT{p}")
                qps = [
                    qkps.tile([128, 512], f32, tag="qkps", name=f"qps{p}_{m}")
                    for m in range(4)
                ]
                kps = [
                    qkps.tile([128, 512], f32, tag="qkps", name=f"kps{p}_{m}")
                    for m in range(4)
                ]
                for k in range(KT):
                    xk = xq.tile([128, T], f32r, tag="xk2", name=f"xk2_{p}_{k}")
                    nc.sync.dma_start(xk[:], xT[k * 128:(k + 1) * 128, :])
                    for m in range(4):
                        nc.tensor.matmul(
                            qps[m][:],
                            lhsT=(wq_sb[:, k, :]),
                            rhs=(xk[:, m * 512:(m + 1) * 512]),
                            start=(k == 0),
                            stop=(k == KT - 1),
                        )
                        nc.tensor.matmul(
                            kps[m][:],
                            lhsT=(wk_sb[:, k, :]),
                            rhs=(xk[:, m * 512:(m + 1) * 512]),
                            start=(k == 0),
                            stop=(k == KT - 1),
                        )
                for m in range(4):
                    nc.vector.tensor_scalar_add(
                        qT[:, m * 512:(m + 1) * 512], qps[m][:], bq_sb[:, p:p + 1]
                    )
                    nc.vector.tensor_scalar_add(
                        kTt[:, m * 512:(m + 1) * 512], kps[m][:], bk_sb[:, p:p + 1]
                    )

            # ---------------- Attention for the two heads of the pair -------
            with tc.tile_pool(name=f"sps{p}", bufs=2, space="PSUM") as sps, \
                 tc.tile_pool(name=f"yps{p}", bufs=4, space="PSUM") as yps:
                for hh in range(2):
                    h = p * 2 + hh
                    hs = slice(hh * 64, hh * 64 + 64)
                    ypt = [
                        yps.tile([128, 512], f32, tag="yps", name=f"y{p}_{hh}_{ic}")
                        for ic in range(4)
                    ]
                    for j in range(MT):
                        W = T - 128 * j
                        PT = ptp.tile([128, T], f32r, tag="pt", name=f"pt{p}_{hh}_{j}")
                        for s in range((W + 1023) // 1024):
                            sw = min(1024, W - s * 1024)
                            ps = sps.tile(
                                [128, 1024], f32, tag="sps", name=f"s{p}_{hh}_{j}_{s}"
                            )
                            for half in range((sw + 511) // 512):
                                w = min(512, sw - half * 512)
                                io = 128 * j + s * 1024 + half * 512
                                nc.tensor.matmul(
                                    ps[:, half * 512:half * 512 + w],
                                    lhsT=(kTt[hs, j * 128:(j + 1) * 128]),
                                    rhs=(qT[hs, io:io + w]),
                                    start=True,
                                    stop=True,
                                )
                            nc.scalar.activation(
                                PT[:, s * 1024:s * 1024 + sw],
                                ps[:, 0:sw],
                                AF.Exp,
                                scale=SC,
                            )
                        # zero the upper-triangular part of the diagonal block
                        nc.vector.tensor_mul(
                            PT[:, 0:128], PT[:, 0:128], mask_tri[:]
                        )
                        for ic in range(j // 4, 4):
                            a = max(ic * 512, 128 * j)
                            w = (ic + 1) * 512 - a
                            nc.tensor.matmul(
                                ypt[ic][0:65, a - ic * 512:512],
                                lhsT=(v4[:, j, h, :]),
                                rhs=(PT[:, a - 128 * j:a - 128 * j + w]),
                                start=(j == 0),
                                stop=(j == 4 * ic + 3),
                            )
                    for ic in range(4):
                        sums = nrm.tile([1, 512], f32, tag="sums",
                                        name=f"sm{p}_{hh}_{ic}")
                        nc.vector.tensor_copy(sums[:], ypt[ic][64:65, :])
                        bc = nrm.tile([64, 512], f32, tag="bc",
                                      name=f"bc{p}_{hh}_{ic}")
                        nc.gpsimd.partition_broadcast(bc[:], sums[:])
                        rcp = nrm.tile([64, 512], f32, tag="rcp",
                                       name=f"rc{p}_{hh}_{ic}")
                        nc.vector.reciprocal_approx_fast(rcp[:], bc[:])
                        nc.vector.tensor_mul(
                            yT_tiles[p][hs, ic * 512:(ic + 1) * 512],
                            ypt[ic][0:64, :],
                            rcp[:],
                        )

        # ---------------- Output projection (partial over 512 channels) -----
        with tc.tile_pool(name="wpp", bufs=1) as wpp, \
             tc.tile_pool(name="ops", bufs=4, space="PSUM") as ops, \
             tc.tile_pool(name="ost", bufs=3) as ostp:
            wp_sb = wpp.tile([128, 4, C], f32r)
            nc.sync.dma_start(wp_sb[:], wpD.rearrange("(k p) n -> p k n", p=128))
            for mt in range(MT):
                ost = ostp.tile([128, C], f32, tag="ost", name=f"ost{mt}")
                for oh in range(2):
                    pps = ops.tile([128, 512], f32, tag="ops", name=f"pp{mt}_{oh}")
                    for k in range(4):
                        nc.tensor.matmul(
                            pps[:],
                            lhsT=(yT_tiles[k][:, mt * 128:(mt + 1) * 128]),
                            rhs=(wp_sb[:, k, oh * 512:(oh + 1) * 512]),
                            start=(k == 0),
                            stop=(k == 3),
                        )
                    nc.vector.tensor_copy(ost[:, oh * 512:(oh + 1) * 512], pps[:])
                nc.sync.dma_start(outD[mt * 128:(mt + 1) * 128, :], ost[:])

    nc.compile()
    return nc


def _get_nc():
    if "nc" not in _CACHE:
        _CACHE["nc"] = _build_nc()
    return _CACHE["nc"]


def make_in_maps(x, Wq, bq, Wk, bk, Wv, bv, Wp, bp):
    x = np.asarray(x, np.float32)
    Wq = np.asarray(Wq, np.float32)
    Wk = np.asarray(Wk, np.float32)
    Wv = np.asarray(Wv, np.float32)
    Wp = np.asarray(Wp, np.float32)
    bq = np.asarray(bq, np.float32)
    bk = np.asarray(bk, np.float32)
    bv = np.asarray(bv, np.float32)
    in_maps = []
    for c in range(NCORES):
        b, hg = divmod(c, 2)
        sl = slice(hg * 512, (hg + 1) * 512)
        in_maps.append({
            "xT": np.ascontiguousarray(x[b].T),
            "wq": np.ascontiguousarray(Wq[:, sl]),
            "wk": np.ascontiguousarray(Wk[:, sl]),
            "wv": np.ascontiguousarray(Wv[:, sl]),
            "wp": np.ascontiguousarray(Wp[sl, :]),
            "bq": np.ascontiguousarray(bq[sl]),
            "bk": np.ascontiguousarray(bk[sl]),
            "bv": np.ascontiguousarray(bv[sl]),
        })
    return in_maps


def combine(results, bp):
    bp = np.asarray(bp, np.float32)
    out = np.empty((B, T, C), np.float32)
    for b in range(B):
        out[b] = results[2 * b]["out"] + results[2 * b + 1]["out"] + bp
    return out


def kernel(x, Wq, bq, Wk, bk, Wv, bv, Wp, bp):
    from concourse import bass_utils

    nc = _get_nc()
    in_maps = make_in_maps(x, Wq, bq, Wk, bk, Wv, bv, Wp, bp)
    res = bass_utils.run_bass_kernel_spmd(nc, in_maps, core_ids=list(range(NCORES)))
    return combine(res.results, bp)


# revision 4
# speedup vs baseline: 1.1885x; 1.1885x over previous
"""Causal self-attention (B=4, T=2048, C=1024, H=16, D=64) on 8 trn2 NeuronCores.

Sharding: core c = (batch b=c//2, head-group hg=c%2 of 8 heads / 512 channels).
Each core computes attention for its 8 heads on its batch plus the partial
output projection over its 512 channels of Wp; the host sums the two partial
projections per batch and adds bp.

Per-core layout is feature-major ("transposed"): x is sent as xT (C, T) so
q/k project directly as qT = Wq.T @ x.T with both operands k(partition)-major.
v is computed in natural (T, D) orientation with a ones-column appended per
head so that the yT = [v|1].T @ P^T matmul also yields softmax row sums.
Matmul operands are bf16 (1 cyc/row on the PE); accumulation, softmax
internals and the final output stay fp32.
"""

import math

import numpy as np

B, T, C = 4, 2048, 1024
H, D = 16, 64
NCORES = 8
PAIRS = 4          # head pairs per core (2 heads = 128 channels each)
KT = C // 128      # 8 k-tiles over input channels
MT = T // 128      # 16 tiles over sequence
SC = 1.0 / math.sqrt(D)

_CACHE = {}


def _build_nc():
    from contextlib import ExitStack

    import concourse.bacc as bacc
    import concourse.mybir as mybir
    import concourse.tile as tile

    f32 = mybir.dt.float32
    bf16 = mybir.dt.bfloat16
    AF = mybir.ActivationFunctionType

    nc = bacc.Bacc("TRN2", target_bir_lowering=False, debug=False)

    xT = nc.dram_tensor("xT", (C, T), bf16, kind="ExternalInput").ap()
    wqD = nc.dram_tensor("wq", (C, 512), bf16, kind="ExternalInput").ap()
    wkD = nc.dram_tensor("wk", (C, 512), bf16, kind="ExternalInput").ap()
    wvD = nc.dram_tensor("wv", (C, 512), bf16, kind="ExternalInput").ap()
    wpD = nc.dram_tensor("wp", (512, C), bf16, kind="ExternalInput").ap()
    bqD = nc.dram_tensor("bq", (512,), f32, kind="ExternalInput").ap()
    bkD = nc.dram_tensor("bk", (512,), f32, kind="ExternalInput").ap()
    bvD = nc.dram_tensor("bv", (512,), f32, kind="ExternalInput").ap()
    outD = nc.dram_tensor("out", (T, C), f32, kind="ExternalOutput").ap()

    with tile.TileContext(nc) as tc, ExitStack() as ctx:
        const = ctx.enter_context(tc.tile_pool(name="const", bufs=1))

        # 128x128 lower-block mask: keep (1.0) where i >= j, else 0.
        mask_tri = const.tile([128, 128], bf16)
        nc.gpsimd.memset(mask_tri[:], 1.0)
        nc.gpsimd.affine_select(
            out=mask_tri[:],
            in_=mask_tri[:],
            compare_op=mybir.AluOpType.is_ge,
            fill=0.0,
            base=0,
            pattern=[[1, 128]],
            channel_multiplier=-1,
        )

        bq_sb = const.tile([128, PAIRS], f32)
        nc.sync.dma_start(bq_sb[:], bqD.rearrange("(a p) -> p a", p=128))
        bk_sb = const.tile([128, PAIRS], f32)
        nc.sync.dma_start(bk_sb[:], bkD.rearrange("(a p) -> p a", p=128))
        bv_row = const.tile([1, 512], f32)
        nc.sync.dma_start(bv_row[:], bvD.rearrange("(a n) -> a n", a=1))
        bv_bc = const.tile([128, 512], f32)
        nc.gpsimd.partition_broadcast(bv_bc[:], bv_row[:])

        # x^T resident in SBUF (bf16, 32KB/part), loaded once.
        xsb = [const.tile([128, T], bf16, name=f"xsb{k}") for k in range(KT)]
        for k in range(KT):
            nc.sync.dma_start(xsb[k][:], xT[k * 128:(k + 1) * 128, :])

        # v for all heads, natural (t, d) layout, 65-wide blocks per head with
        # a ones column at index 64 (memset 1.0; evictions fill cols 0:64).
        v_all = const.tile([128, MT * 8 * 65], bf16)
        nc.gpsimd.memset(v_all[:], 1.0)
        v4 = v_all.rearrange("p (t h e) -> p t h e", t=MT, h=8)

        yT_tiles = [const.tile([128, T], bf16, name=f"yT{i}") for i in range(PAIRS)]

        # ---------------- Phase V: v = x @ Wv + bv for all 8 heads ----------
        with tc.tile_pool(name="wvp", bufs=1) as wvp, \
             tc.tile_pool(name="vps", bufs=8, space="PSUM") as vps:
            wv_sb = wvp.tile([128, KT, 512], bf16)
            nc.sync.dma_start(wv_sb[:], wvD.rearrange("(k p) n -> p k n", p=128))
            for tg in range(2):
                ps = [
                    vps.tile([128, 512], f32, tag="vps", name=f"vps{tg}_{t}")
                    for t in range(8)
                ]
                for k in range(KT):
                    for t8 in range(8):
                        tt = tg * 8 + t8
                        nc.tensor.matmul(
                            ps[t8][:],
                            lhsT=xsb[k][:, tt * 128:(tt + 1) * 128],
                            rhs=wv_sb[:, k, :],
                            start=(k == 0),
                            stop=(k == KT - 1),
                        )
                for t8 in range(8):
                    tt = tg * 8 + t8
                    nc.vector.tensor_add(
                        v4[:, tt, :, 0:64],
                        ps[t8].rearrange("p (h e) -> p h e", h=8),
                        bv_bc.rearrange("p (h e) -> p h e", h=8),
                    )

        qkbuf = ctx.enter_context(tc.tile_pool(name="qkbuf", bufs=2))
        wqk = ctx.enter_context(tc.tile_pool(name="wqk", bufs=2))
        ptp = ctx.enter_context(tc.tile_pool(name="ptp", bufs=3))
        nrm = ctx.enter_context(tc.tile_pool(name="nrm", bufs=2))

        for p in range(PAIRS):
            # ---------------- QK projections for this head pair -------------
            with tc.tile_pool(name=f"qkps{p}", bufs=8, space="PSUM") as qkps:
                wq_sb = wqk.tile([128, KT, 128], bf16, tag="wq", name=f"wq{p}")
                nc.sync.dma_start(
                    wq_sb[:],
                    wqD[:, p * 128:(p + 1) * 128].rearrange("(k q) n -> q k n", q=128),
                )
                wk_sb = wqk.tile([128, KT, 128], bf16, tag="wk", name=f"wk{p}")
                nc.sync.dma_start(
                    wk_sb[:],
                    wkD[:, p * 128:(p + 1) * 128].rearrange("(k q) n -> q k n", q=128),
                )
                qT = qkbuf.tile([128, T], bf16, tag="qT", name=f"qT{p}")
                kTt = qkbuf.tile([128, T], bf16, tag="kT", name=f"kT{p}")
                qps = [
                    qkps.tile([128, 512], f32, tag="qkps", name=f"qps{p}_{m}")
                    for m in range(4)
                ]
                kps = [
                    qkps.tile([128, 512], f32, tag="qkps", name=f"kps{p}_{m}")
                    for m in range(4)
                ]
                for k in range(KT):
                    for m in range(4):
                        nc.tensor.matmul(
                            qps[m][:],
                            lhsT=wq_sb[:, k, :],
                            rhs=xsb[k][:, m * 512:(m + 1) * 512],
                            start=(k == 0),
                            stop=(k == KT - 1),
                        )
                        nc.tensor.matmul(
                            kps[m][:],
                            lhsT=wk_sb[:, k, :],
                            rhs=xsb[k][:, m * 512:(m + 1) * 512],
                            start=(k == 0),
                            stop=(k == KT - 1),
                        )
                for m in range(4):
                    nc.vector.tensor_scalar_add(
                        qT[:, m * 512:(m + 1) * 512], qps[m][:], bq_sb[:, p:p + 1]
                    )
                    nc.vector.tensor_scalar_add(
                        kTt[:, m * 512:(m + 1) * 512], kps[m][:], bk_sb[:, p:p + 1]
                    )

            # ---------------- Attention for the two heads of the pair -------
            with tc.tile_pool(name=f"sps{p}", bufs=2, space="PSUM") as sps, \
                 tc.tile_pool(name=f"yps{p}", bufs=4, space="PSUM") as yps:
                for hh in range(2):
                    h = p * 2 + hh
                    hs = slice(hh * 64, hh * 64 + 64)
                    ypt = [
                        yps.tile([128, 512], f32, tag="yps", name=f"y{p}_{hh}_{ic}")
                        for ic in range(4)
                    ]
                    for j in range(MT):
                        W = T - 128 * j
                        PT = ptp.tile([128, T], bf16, tag="pt", name=f"pt{p}_{hh}_{j}")
                        for s in range((W + 1023) // 1024):
                            sw = min(1024, W - s * 1024)
                            ps = sps.tile(
                                [128, 1024], f32, tag="sps", name=f"s{p}_{hh}_{j}_{s}"
                            )
                            for half in range((sw + 511) // 512):
                                w = min(512, sw - half * 512)
                                io = 128 * j + s * 1024 + half * 512
                                nc.tensor.matmul(
                                    ps[:, half * 512:half * 512 + w],
                                    lhsT=kTt[hs, j * 128:(j + 1) * 128],
                                    rhs=qT[hs, io:io + w],
                                    start=True,
                                    stop=True,
                                )
                            nc.scalar.activation(
                                PT[:, s * 1024:s * 1024 + sw],
                                ps[:, 0:sw],
                                AF.Exp,
                                scale=SC,
                            )
                        # zero the upper-triangular part of the diagonal block
                        nc.vector.tensor_mul(
                            PT[:, 0:128], PT[:, 0:128], mask_tri[:]
                        )
                        for ic in range(j // 4, 4):
                            a = max(ic * 512, 128 * j)
                            w = (ic + 1) * 512 - a
                            nc.tensor.matmul(
                                ypt[ic][0:65, a - ic * 512:512],
                                lhsT=v4[:, j, h, :],
                                rhs=PT[:, a - 128 * j:a - 128 * j + w],
                                start=(j == 0),
                                stop=(j == 4 * ic + 3),
                            )
                    for ic in range(4):
                        sums = nrm.tile([1, 512], f32, tag="sums",
                                        name=f"sm{p}_{hh}_{ic}")
                        nc.vector.tensor_copy(sums[:], ypt[ic][64:65, :])
                        bc = nrm.tile([64, 512], f32, tag="bc",
                                      name=f"bc{p}_{hh}_{ic}")
                        nc.gpsimd.partition_broadcast(bc[:], sums[:])
                        rcp = nrm.tile([64, 512], f32, tag="rcp",
                                       name=f"rc{p}_{hh}_{ic}")
                        nc.vector.reciprocal_approx_fast(rcp[:], bc[:])
                        nc.vector.tensor_mul(
                            yT_tiles[p][hs, ic * 512:(ic + 1) * 512],
                            ypt[ic][0:64, :],
                            rcp[:],
                        )

        # ---------------- Output projection (partial over 512 channels) -----
        with tc.tile_pool(name="wpp", bufs=1) as wpp, \
             tc.tile_pool(name="ops", bufs=4, space="PSUM") as ops, \
             tc.tile_pool(name="ost", bufs=3) as ostp:
            wp_sb = wpp.tile([128, 4, C], bf16)
            nc.sync.dma_start(wp_sb[:], wpD.rearrange("(k p) n -> p k n", p=128))
            for mt in range(MT):
                ost = ostp.tile([128, C], f32, tag="ost", name=f"ost{mt}")
                for oh in range(2):
                    pps = ops.tile([128, 512], f32, tag="ops", name=f"pp{mt}_{oh}")
                    for k in range(4):
                        nc.tensor.matmul(
                            pps[:],
                            lhsT=yT_tiles[k][:, mt * 128:(mt + 1) * 128],
                            rhs=wp_sb[:, k, oh * 512:(oh + 1) * 512],
                            start=(k == 0),
                            stop=(k == 3),
                        )
                    nc.vector.tensor_copy(ost[:, oh * 512:(oh + 1) * 512], pps[:])
                nc.sync.dma_start(outD[mt * 128:(mt + 1) * 128, :], ost[:])

    nc.compile()
    return nc


def _get_nc():
    if "nc" not in _CACHE:
        _CACHE["nc"] = _build_nc()
    return _CACHE["nc"]


def make_in_maps(x, Wq, bq, Wk, bk, Wv, bv, Wp, bp):
    import ml_dtypes

    bf = ml_dtypes.bfloat16
    x = np.asarray(x, np.float32)
    Wq = np.asarray(Wq, np.float32).astype(bf)
    Wk = np.asarray(Wk, np.float32).astype(bf)
    Wv = np.asarray(Wv, np.float32).astype(bf)
    Wp = np.asarray(Wp, np.float32).astype(bf)
    bq = np.asarray(bq, np.float32)
    bk = np.asarray(bk, np.float32)
    bv = np.asarray(bv, np.float32)
    in_maps = []
    for c in range(NCORES):
        b, hg = divmod(c, 2)
        sl = slice(hg * 512, (hg + 1) * 512)
        in_maps.append({
            "xT": np.ascontiguousarray(x[b].T.astype(bf)),
            "wq": np.ascontiguousarray(Wq[:, sl]),
            "wk": np.ascontiguousarray(Wk[:, sl]),
            "wv": np.ascontiguousarray(Wv[:, sl]),
            "wp": np.ascontiguousarray(Wp[sl, :]),
            "bq": np.ascontiguousarray(bq[sl]),
            "bk": np.ascontiguousarray(bk[sl]),
            "bv": np.ascontiguousarray(bv[sl]),
        })
    return in_maps


def combine(results, bp):
    bp = np.asarray(bp, np.float32)
    out = np.empty((B, T, C), np.float32)
    for b in range(B):
        out[b] = results[2 * b]["out"] + results[2 * b + 1]["out"] + bp
    return out


def kernel(x, Wq, bq, Wk, bk, Wv, bv, Wp, bp):
    from concourse import bass_utils

    nc = _get_nc()
    in_maps = make_in_maps(x, Wq, bq, Wk, bk, Wv, bv, Wp, bp)
    res = bass_utils.run_bass_kernel_spmd(nc, in_maps, core_ids=list(range(NCORES)))
    return combine(res.results, bp)
